# revision 13
# baseline (speedup 1.0000x reference)
"""DLRM forward kernel for 8 Trainium2 NeuronCores (Bass/Tile) — fp8 version.

Sharding: data-parallel batch split 8 ways (2048 rows/core). The 26
embedding tables are flattened into one [26*100000, 128] DRAM tensor,
pre-scaled by S_E=64 and quantized to fp8e4 on the host, and replicated
on every core (no collective needed).

Per core:
  - one indirect DMA per (128-sample tile, table) gathers the bf16
    embedding rows (indices pre-offset by table*VOCAB on the host)
  - PE transpose-mode (bf16, 1 cycle/row) turns each gathered
    [sample, feat] block into feature-major [feat, sample]; the
    PSUM->SBUF copy quantizes to fp8 (table is pre-scaled by S_E)
  - the 4-layer MLP runs feature-major with all weights resident in
    SBUF. L1/L2/L3 use fp8e4 DoubleRow matmuls (2 contraction blocks
    per instruction, 0.5 cycles/row) with host-side scaling:
      emb*64 -> fp8, W1e*64 -> fp8, W2*64 -> fp8, W3*64 -> fp8,
      h1 = relu(ps/64' ...) stored as 64*h1 in fp8, h2 likewise;
    scales cancel via the activation `scale` argument. The continuous
    bottom MLP is folded into layer 1 (wx = W_cont @ W1[:128], bf16,
    carrying the 64*64 product scale); the output layer runs bf16.
    Measured accuracy vs fp32 reference: absmax rel err ~8e-3.
  - accumulation is always fp32 PSUM.

`repeat` wraps the whole per-core program in a hardware loop for
benchmarking (identical output every iteration).
"""

import os
import sys

import numpy as np

for _p in ("/opt/trn_rl_repo",):
    if os.path.isdir(_p) and _p not in sys.path:
        sys.path.insert(0, _p)

import ml_dtypes

N_TABLES = 26
VOCAB = 100000
D = 128
N_CONT = 13
N_CONT_PAD = 32
BATCH = 16384
L1, L2, L3 = 1024, 512, 256
N_CORES = 8
P = 128

S_E = 64.0    # embedding scale (fp8)
S_W = 64.0    # weight scale for W1e/W2/W3 (fp8)
S_H = 64.0    # activation scale for h1/h2 (fp8 storage)
FP8_MAX = 240.0


def build_nc(vocab=VOCAB, shard=BATCH // N_CORES, chunk=512, repeat=1):
    import contextlib

    import concourse.bass as bass
    import concourse.mybir as mybir
    from concourse import bacc, tile
    from concourse.masks import make_identity

    f32 = mybir.dt.float32
    bf16 = mybir.dt.bfloat16
    fp8 = mybir.dt.float8e4
    i32 = mybir.dt.int32
    AF = mybir.ActivationFunctionType
    DR = mybir.MatmulPerfMode.DoubleRow

    n_chunks = shard // chunk
    bt_per_chunk = chunk // P
    n_bt = shard // P

    nc = bacc.Bacc(None, target_bir_lowering=False, debug=False)

    emb = nc.dram_tensor("emb", [N_TABLES * vocab, D], bf16, kind="ExternalInput")
    idx = nc.dram_tensor("idx", [P, n_bt * N_TABLES], i32, kind="ExternalInput")
    xT = nc.dram_tensor("xT", [N_CONT_PAD, shard], bf16, kind="ExternalInput")
    w1e = nc.dram_tensor("w1e", [P, N_TABLES, L1], fp8, kind="ExternalInput")
    wx = nc.dram_tensor("wx", [N_CONT_PAD, L1], bf16, kind="ExternalInput")
    w2 = nc.dram_tensor("w2", [P, L1 // P, L2], fp8, kind="ExternalInput")
    w3 = nc.dram_tensor("w3", [P, L2 // P, L3], fp8, kind="ExternalInput")
    wo = nc.dram_tensor("wo", [P, L3 // P], bf16, kind="ExternalInput")
    b1 = nc.dram_tensor("b1", [P, L1 // P], f32, kind="ExternalInput")
    b2 = nc.dram_tensor("b2", [P, L2 // P], f32, kind="ExternalInput")
    b3 = nc.dram_tensor("b3", [P, L3 // P], f32, kind="ExternalInput")
    bo = nc.dram_tensor("bo", [1, 1], f32, kind="ExternalInput")
    y = nc.dram_tensor("y", [n_chunks, chunk], f32, kind="ExternalOutput")

    with tile.TileContext(nc) as tc:
        with (
            tc.tile_pool(name="cpool", bufs=1) as cpool,
            tc.tile_pool(name="epool", bufs=3) as epool,
            tc.tile_pool(name="h0pool", bufs=1) as h0pool,
            tc.tile_pool(name="hpool", bufs=1) as hpool,
            tc.tile_pool(name="xpool", bufs=2) as xpool,
            tc.tile_pool(name="opool", bufs=2) as opool,
            tc.tile_pool(name="psA", bufs=2, space="PSUM") as psA,
            tc.tile_pool(name="psT", bufs=4, space="PSUM") as psT,
            tc.tile_pool(name="psO", bufs=1, space="PSUM") as psO,
        ):
            ident = cpool.tile([P, P], bf16, name="ident")
            make_identity(nc, ident[:])
            idx_sb = cpool.tile_from(idx[:, :], name="idx_sb")
            w1e_sb = cpool.tile_from(w1e[:, :, :], name="w1e_sb")
            wx_sb = cpool.tile_from(wx[:, :], name="wx_sb")
            w2_sb = cpool.tile_from(w2[:, :, :], name="w2_sb")
            w3_sb = cpool.tile_from(w3[:, :, :], name="w3_sb")
            wo_sb = cpool.tile_from(wo[:, :], name="wo_sb")
            b1_sb = cpool.tile_from(b1[:, :], name="b1_sb")
            b2_sb = cpool.tile_from(b2[:, :], name="b2_sb")
            b3_sb = cpool.tile_from(b3[:, :], name="b3_sb")
            bo_sb = cpool.tile_from(bo[:, :], name="bo_sb")

            loop_ctx = (
                tc.For_i(0, repeat, 1) if repeat > 1 else contextlib.nullcontext()
            )
            with loop_ctx:
              for c in range(n_chunks):
                xc = xpool.tile([N_CONT_PAD, chunk], bf16, name="xc", tag="xc")
                nc.sync.dma_start(out=xc[:], in_=xT[:, c * chunk : (c + 1) * chunk])

                h0T = h0pool.tile(
                    [P, N_TABLES, chunk], fp8, name="h0T", tag="h0T"
                )
                for bt in range(bt_per_chunk):
                    g = c * bt_per_chunk + bt
                    e_t = epool.tile([P, N_TABLES * D], bf16, name="e_t", tag="E")
                    # HW indirect DMA applies ONE index per partition per
                    # instruction (the offset AP's free dim selects
                    # consecutive rows instead), so gather one table per
                    # instruction.
                    for t in range(N_TABLES):
                        nc.gpsimd.indirect_dma_start(
                            out=e_t[:, t * D : (t + 1) * D],
                            out_offset=None,
                            in_=emb[:],
                            in_offset=bass.IndirectOffsetOnAxis(
                                ap=idx_sb[
                                    :, g * N_TABLES + t : g * N_TABLES + t + 1
                                ],
                                axis=0,
                            ),
                        )
                    for t in range(N_TABLES):
                        tp = psT.tile([P, P], bf16, name="tp", tag="tp")
                        nc.tensor.transpose(
                            tp[:], e_t[:, t * D : (t + 1) * D], ident[:]
                        )
                        nc.any.tensor_copy(
                            h0T[:, t, bt * P : (bt + 1) * P],
                            tp[:],
                        )

                # L1: fp8 DoubleRow over 13 table pairs + bf16 cont block.
                h1T = hpool.tile([P, L1 // P, chunk], fp8, name="h1T", tag="h1")
                for m in range(L1 // P):
                    ps1 = psA.tile([P, chunk], f32, name="ps1", tag="mm")
                    nc.tensor.matmul(
                        ps1[:],
                        wx_sb[:, m * P : (m + 1) * P],
                        xc[:],
                        start=True,
                        stop=False,
                    )
                    for tp2 in range(N_TABLES // 2):
                        nc.tensor.matmul(
                            ps1[:],
                            w1e_sb[:, 2 * tp2 : 2 * tp2 + 2, m * P : (m + 1) * P],
                            h0T[:, 2 * tp2 : 2 * tp2 + 2, :],
                            start=False,
                            stop=(tp2 == N_TABLES // 2 - 1),
                            perf_mode=DR,
                        )
                    # psum carries (S_E*S_W) * z1; store S_H * relu(z1).
                    nc.scalar.activation(
                        h1T[:, m, :],
                        ps1[:],
                        AF.Relu,
                        bias=b1_sb[:, m : m + 1],
                        scale=S_H / (S_E * S_W),
                    )

                h2T = hpool.tile([P, L2 // P, chunk], fp8, name="h2T", tag="h2")
                for m in range(L2 // P):
                    ps2 = psA.tile([P, chunk], f32, name="ps2", tag="mm")
                    for j in range(0, L1 // P, 2):
                        nc.tensor.matmul(
                            ps2[:],
                            w2_sb[:, j : j + 2, m * P : (m + 1) * P],
                            h1T[:, j : j + 2, :],
                            start=(j == 0),
                            stop=(j == L1 // P - 2),
                            perf_mode=DR,
                        )
                    nc.scalar.activation(
                        h2T[:, m, :],
                        ps2[:],
                        AF.Relu,
                        bias=b2_sb[:, m : m + 1],
                        scale=S_H / (S_W * S_H),
                    )

                h3T = hpool.tile([P, L3 // P, chunk], bf16, name="h3T", tag="h3")
                for m in range(L3 // P):
                    ps3 = psA.tile([P, chunk], f32, name="ps3", tag="mm")
                    for j in range(0, L2 // P, 2):
                        nc.tensor.matmul(
                            ps3[:],
                            w3_sb[:, j : j + 2, m * P : (m + 1) * P],
                            h2T[:, j : j + 2, :],
                            start=(j == 0),
                            stop=(j == L2 // P - 2),
                            perf_mode=DR,
                        )
                    nc.scalar.activation(
                        h3T[:, m, :],
                        ps3[:],
                        AF.Relu,
                        bias=b3_sb[:, m : m + 1],
                        scale=1.0 / (S_W * S_H),
                    )

                pso = psO.tile([1, chunk], f32, name="pso", tag="out")
                for j in range(L3 // P):
                    nc.tensor.matmul(
                        pso[:],
                        wo_sb[:, j : j + 1],
                        h3T[:, j, :],
                        start=(j == 0),
                        stop=(j == L3 // P - 1),
                    )
                yo = opool.tile([1, chunk], f32, name="yo", tag="yo")
                nc.scalar.activation(
                    yo[:], pso[:], AF.Sigmoid, bias=bo_sb[0:1, 0:1]
                )
                nc.sync.dma_start(out=y[c : c + 1, :], in_=yo[:])

    nc.compile()
    return nc


def _q8(x, scale):
    y = np.clip(np.asarray(x, np.float32) * scale, -FP8_MAX, FP8_MAX)
    return y.astype(ml_dtypes.float8_e4m3)


def prep_shared(inputs, vocab=VOCAB):
    """Host-side weight/table prep shared by all cores."""
    emb = (
        np.asarray(inputs["emb_tables"], np.float32).reshape(N_TABLES * vocab, D)
        * S_E
    ).astype(ml_dtypes.bfloat16)  # fp8 quantization happens at the h0T copy
    W_cont = np.asarray(inputs["W_cont"], np.float64)
    b_cont = np.asarray(inputs["b_cont"], np.float64)
    W1 = np.asarray(inputs["W1"], np.float32)
    b1 = np.asarray(inputs["b1"], np.float32)
    W2 = np.asarray(inputs["W2"], np.float32)
    b2 = np.asarray(inputs["b2"], np.float32)
    W3 = np.asarray(inputs["W3"], np.float32)
    b3 = np.asarray(inputs["b3"], np.float32)
    W_out = np.asarray(inputs["W_out"], np.float32)
    b_out = np.asarray(inputs["b_out"], np.float32)

    W1x = W1[:D].astype(np.float64)  # [128, 1024] rows feeding the dense bottom
    wx13 = (W_cont @ W1x) * (S_E * S_W)  # carry the fp8 product scale
    wx = np.zeros((N_CONT_PAD, L1), np.float32)
    wx[:N_CONT] = wx13.astype(np.float32)
    wx = np.ascontiguousarray(wx).astype(ml_dtypes.bfloat16)
    b1f = (b1.astype(np.float64) + b_cont @ W1x).astype(np.float32) * S_H
    W1e = W1[D:]  # [3328, 1024]

    w1e_sb = _q8(
        np.ascontiguousarray(
            W1e.reshape(N_TABLES, P, L1).transpose(1, 0, 2)
        ),
        S_W,
    )
    w2_sb = _q8(
        np.ascontiguousarray(W2.reshape(L1 // P, P, L2).transpose(1, 0, 2)), S_W
    )
    w3_sb = _q8(
        np.ascontiguousarray(W3.reshape(L2 // P, P, L3).transpose(1, 0, 2)), S_W
    )
    wo_sb = np.ascontiguousarray(
        W_out.reshape(L3 // P, P, 1).transpose(1, 0, 2).reshape(P, L3 // P)
    ).astype(ml_dtypes.bfloat16)
    b1_sb = np.ascontiguousarray(b1f.reshape(L1 // P, P).T)
    b2_sb = np.ascontiguousarray((b2 * S_H).reshape(L2 // P, P).T)
    b3_sb = np.ascontiguousarray(b3.reshape(L3 // P, P).T)
    bo_sb = np.ascontiguousarray(b_out.reshape(1, 1))

    return dict(
        emb=emb, w1e=w1e_sb, wx=wx, w2=w2_sb, w3=w3_sb, wo=wo_sb,
        b1=b1_sb, b2=b2_sb, b3=b3_sb, bo=bo_sb,
    )


def prep_core(inputs, core, vocab=VOCAB, shard=BATCH // N_CORES):
    """Per-core shard of indices and continuous features."""
    cf = np.asarray(inputs["continuous_features"], np.float32)
    cat = np.asarray(inputs["categorical_features"])
    n_bt = shard // P
    offs = (np.arange(N_TABLES, dtype=np.int64) * vocab)[None, :]
    sl = slice(core * shard, (core + 1) * shard)
    gidx = (cat[sl] + offs).astype(np.int32)  # [shard, 26]
    idx_c = np.ascontiguousarray(
        gidx.reshape(n_bt, P, N_TABLES).transpose(1, 0, 2).reshape(P, n_bt * N_TABLES)
    )
    xT_c = np.zeros((N_CONT_PAD, shard), np.float32)
    xT_c[:N_CONT] = cf[sl].T
    xT_c = np.ascontiguousarray(xT_c).astype(ml_dtypes.bfloat16)
    return dict(idx=idx_c, xT=xT_c)


_CACHE = {}


def get_nc(repeat=1):
    key = ("nc", repeat)
    if key not in _CACHE:
        _CACHE[key] = build_nc(repeat=repeat)
    return _CACHE[key]


def run(inputs, trace=False, repeat=1, **spmd_kwargs):
    from concourse.bass_utils import run_bass_kernel_spmd

    nc = get_nc(repeat)

    shared = prep_shared(inputs)
    in_maps = []
    for c in range(N_CORES):
        m = dict(shared)
        m.update(prep_core(inputs, c))
        in_maps.append(m)

    res = run_bass_kernel_spmd(
        nc, in_maps, core_ids=list(range(N_CORES)), trace=trace, **spmd_kwargs
    )
    ys = [r["y"].reshape(-1) for r in res.results]
    out = np.concatenate(ys).reshape(BATCH, 1).astype(np.float32)
    return out, res


def kernel(**inputs):
    out, _ = run(inputs, trace=False)
    return out


# revision 14
# speedup vs baseline: 1.1213x; 1.1213x over previous
"""DLRM forward kernel for 8 Trainium2 NeuronCores (Bass/Tile) — fp8 version.

Sharding: data-parallel batch split 8 ways (2048 rows/core). The 26
embedding tables are flattened into one [26*100000, 128] DRAM tensor,
pre-scaled by S_E=64 and quantized to fp8e4 on the host, and replicated
on every core (no collective needed).

Per core:
  - one indirect DMA per (128-sample tile, table) gathers the bf16
    embedding rows (indices pre-offset by table*VOCAB on the host)
  - PE transpose-mode (bf16, 1 cycle/row) turns each gathered
    [sample, feat] block into feature-major [feat, sample]; the
    PSUM->SBUF copy quantizes to fp8 (table is pre-scaled by S_E)
  - the 4-layer MLP runs feature-major with all weights resident in
    SBUF. L1/L2/L3 use fp8e4 DoubleRow matmuls (2 contraction blocks
    per instruction, 0.5 cycles/row) with host-side scaling:
      emb*64 -> fp8, W1e*64 -> fp8, W2*64 -> fp8, W3*64 -> fp8,
      h1 = relu(ps/64' ...) stored as 64*h1 in fp8, h2 likewise;
    scales cancel via the activation `scale` argument. The continuous
    bottom MLP is folded into layer 1 (wx = W_cont @ W1[:128], bf16,
    carrying the 64*64 product scale); the output layer runs bf16.
    Measured accuracy vs fp32 reference: absmax rel err ~8e-3.
  - accumulation is always fp32 PSUM.

`repeat` wraps the whole per-core program in a hardware loop for
benchmarking (identical output every iteration).
"""

import os
import sys

import numpy as np

for _p in ("/opt/trn_rl_repo",):
    if os.path.isdir(_p) and _p not in sys.path:
        sys.path.insert(0, _p)

import ml_dtypes

N_TABLES = 26
VOCAB = 100000
D = 128
N_CONT = 13
N_CONT_PAD = 32
BATCH = 16384
L1, L2, L3 = 1024, 512, 256
N_CORES = 8
P = 128

S_E = 64.0    # embedding scale (fp8)
S_W = 64.0    # weight scale for W1e/W2/W3 (fp8)
S_H = 64.0    # activation scale for h1/h2 (fp8 storage)
FP8_MAX = 240.0


def build_nc(vocab=VOCAB, shard=BATCH // N_CORES, chunk=512, repeat=1):
    import contextlib

    import concourse.bass as bass
    import concourse.mybir as mybir
    from concourse import bacc, tile
    from concourse.masks import make_identity

    f32 = mybir.dt.float32
    bf16 = mybir.dt.bfloat16
    fp8 = mybir.dt.float8e4
    i32 = mybir.dt.int32
    AF = mybir.ActivationFunctionType
    DR = mybir.MatmulPerfMode.DoubleRow

    n_chunks = shard // chunk
    bt_per_chunk = chunk // P
    n_bt = shard // P

    nc = bacc.Bacc(None, target_bir_lowering=False, debug=False)

    emb = nc.dram_tensor("emb", [N_TABLES * vocab, D], bf16, kind="ExternalInput")
    idx = nc.dram_tensor("idx", [P, n_bt * N_TABLES], i32, kind="ExternalInput")
    xT = nc.dram_tensor("xT", [N_CONT_PAD, shard], bf16, kind="ExternalInput")
    w1e = nc.dram_tensor("w1e", [P, N_TABLES, L1], fp8, kind="ExternalInput")
    wx = nc.dram_tensor("wx", [N_CONT_PAD, L1], bf16, kind="ExternalInput")
    w2 = nc.dram_tensor("w2", [P, L1 // P, L2], fp8, kind="ExternalInput")
    w3 = nc.dram_tensor("w3", [P, L2 // P, L3], fp8, kind="ExternalInput")
    wo = nc.dram_tensor("wo", [P, L3 // P], bf16, kind="ExternalInput")
    b1 = nc.dram_tensor("b1", [P, L1 // P], f32, kind="ExternalInput")
    b2 = nc.dram_tensor("b2", [P, L2 // P], f32, kind="ExternalInput")
    b3 = nc.dram_tensor("b3", [P, L3 // P], f32, kind="ExternalInput")
    bo = nc.dram_tensor("bo", [1, 1], f32, kind="ExternalInput")
    y = nc.dram_tensor("y", [n_chunks, chunk], f32, kind="ExternalOutput")

    with tile.TileContext(nc) as tc:
        with (
            tc.tile_pool(name="cpool", bufs=1) as cpool,
            tc.tile_pool(name="epool", bufs=6) as epool,
            tc.tile_pool(name="h0pool", bufs=2) as h0pool,
            tc.tile_pool(name="hpool", bufs=2) as hpool,
            tc.tile_pool(name="xpool", bufs=2) as xpool,
            tc.tile_pool(name="opool", bufs=2) as opool,
            tc.tile_pool(name="psA", bufs=2, space="PSUM") as psA,
            tc.tile_pool(name="psT", bufs=4, space="PSUM") as psT,
            tc.tile_pool(name="psO", bufs=1, space="PSUM") as psO,
        ):
            ident = cpool.tile([P, P], bf16, name="ident")
            make_identity(nc, ident[:])
            idx_sb = cpool.tile_from(idx[:, :], name="idx_sb")
            w1e_sb = cpool.tile_from(w1e[:, :, :], name="w1e_sb")
            wx_sb = cpool.tile_from(wx[:, :], name="wx_sb")
            w2_sb = cpool.tile_from(w2[:, :, :], name="w2_sb")
            w3_sb = cpool.tile_from(w3[:, :, :], name="w3_sb")
            wo_sb = cpool.tile_from(wo[:, :], name="wo_sb")
            b1_sb = cpool.tile_from(b1[:, :], name="b1_sb")
            b2_sb = cpool.tile_from(b2[:, :], name="b2_sb")
            b3_sb = cpool.tile_from(b3[:, :], name="b3_sb")
            bo_sb = cpool.tile_from(bo[:, :], name="bo_sb")

            loop_ctx = (
                tc.For_i(0, repeat, 1) if repeat > 1 else contextlib.nullcontext()
            )
            with loop_ctx:
              for c in range(n_chunks):
                xc = xpool.tile([N_CONT_PAD, chunk], bf16, name="xc", tag="xc")
                nc.sync.dma_start(out=xc[:], in_=xT[:, c * chunk : (c + 1) * chunk])

                h0T = h0pool.tile(
                    [P, N_TABLES, chunk], fp8, name="h0T", tag="h0T"
                )
                for bt in range(bt_per_chunk):
                    g = c * bt_per_chunk + bt
                    e_t = epool.tile([P, N_TABLES * D], bf16, name="e_t", tag="E")
                    # HW indirect DMA applies ONE index per partition per
                    # instruction (the offset AP's free dim selects
                    # consecutive rows instead), so gather one table per
                    # instruction.
                    for t in range(N_TABLES):
                        nc.gpsimd.indirect_dma_start(
                            out=e_t[:, t * D : (t + 1) * D],
                            out_offset=None,
                            in_=emb[:],
                            in_offset=bass.IndirectOffsetOnAxis(
                                ap=idx_sb[
                                    :, g * N_TABLES + t : g * N_TABLES + t + 1
                                ],
                                axis=0,
                            ),
                        )
                    for t in range(N_TABLES):
                        tp = psT.tile([P, P], bf16, name="tp", tag="tp")
                        nc.tensor.transpose(
                            tp[:], e_t[:, t * D : (t + 1) * D], ident[:]
                        )
                        nc.any.tensor_copy(
                            h0T[:, t, bt * P : (bt + 1) * P],
                            tp[:],
                        )

                # L1: fp8 DoubleRow over 13 table pairs + bf16 cont block.
                h1T = hpool.tile([P, L1 // P, chunk], fp8, name="h1T", tag="h1")
                for m in range(L1 // P):
                    ps1 = psA.tile([P, chunk], f32, name="ps1", tag="mm")
                    nc.tensor.matmul(
                        ps1[:],
                        wx_sb[:, m * P : (m + 1) * P],
                        xc[:],
                        start=True,
                        stop=False,
                    )
                    for tp2 in range(N_TABLES // 2):
                        nc.tensor.matmul(
                            ps1[:],
                            w1e_sb[:, 2 * tp2 : 2 * tp2 + 2, m * P : (m + 1) * P],
                            h0T[:, 2 * tp2 : 2 * tp2 + 2, :],
                            start=False,
                            stop=(tp2 == N_TABLES // 2 - 1),
                            perf_mode=DR,
                        )
                    # psum carries (S_E*S_W) * z1; store S_H * relu(z1).
                    nc.scalar.activation(
                        h1T[:, m, :],
                        ps1[:],
                        AF.Relu,
                        bias=b1_sb[:, m : m + 1],
                        scale=S_H / (S_E * S_W),
                    )

                h2T = hpool.tile([P, L2 // P, chunk], fp8, name="h2T", tag="h2")
                for m in range(L2 // P):
                    ps2 = psA.tile([P, chunk], f32, name="ps2", tag="mm")
                    for j in range(0, L1 // P, 2):
                        nc.tensor.matmul(
                            ps2[:],
                            w2_sb[:, j : j + 2, m * P : (m + 1) * P],
                            h1T[:, j : j + 2, :],
                            start=(j == 0),
                            stop=(j == L1 // P - 2),
                            perf_mode=DR,
                        )
                    nc.scalar.activation(
                        h2T[:, m, :],
                        ps2[:],
                        AF.Relu,
                        bias=b2_sb[:, m : m + 1],
                        scale=S_H / (S_W * S_H),
                    )

                h3T = hpool.tile([P, L3 // P, chunk], bf16, name="h3T", tag="h3")
                for m in range(L3 // P):
                    ps3 = psA.tile([P, chunk], f32, name="ps3", tag="mm")
                    for j in range(0, L2 // P, 2):
                        nc.tensor.matmul(
                            ps3[:],
                            w3_sb[:, j : j + 2, m * P : (m + 1) * P],
                            h2T[:, j : j + 2, :],
                            start=(j == 0),
                            stop=(j == L2 // P - 2),
                            perf_mode=DR,
                        )
                    nc.scalar.activation(
                        h3T[:, m, :],
                        ps3[:],
                        AF.Relu,
                        bias=b3_sb[:, m : m + 1],
                        scale=1.0 / (S_W * S_H),
                    )

                pso = psO.tile([1, chunk], f32, name="pso", tag="out")
                for j in range(L3 // P):
                    nc.tensor.matmul(
                        pso[:],
                        wo_sb[:, j : j + 1],
                        h3T[:, j, :],
                        start=(j == 0),
                        stop=(j == L3 // P - 1),
                    )
                yo = opool.tile([1, chunk], f32, name="yo", tag="yo")
                nc.scalar.activation(
                    yo[:], pso[:], AF.Sigmoid, bias=bo_sb[0:1, 0:1]
                )
                nc.sync.dma_start(out=y[c : c + 1, :], in_=yo[:])

    nc.compile()
    return nc


def _q8(x, scale):
    y = np.clip(np.asarray(x, np.float32) * scale, -FP8_MAX, FP8_MAX)
    return y.astype(ml_dtypes.float8_e4m3)


def prep_shared(inputs, vocab=VOCAB):
    """Host-side weight/table prep shared by all cores."""
    emb = (
        np.asarray(inputs["emb_tables"], np.float32).reshape(N_TABLES * vocab, D)
        * S_E
    ).astype(ml_dtypes.bfloat16)  # fp8 quantization happens at the h0T copy
    W_cont = np.asarray(inputs["W_cont"], np.float64)
    b_cont = np.asarray(inputs["b_cont"], np.float64)
    W1 = np.asarray(inputs["W1"], np.float32)
    b1 = np.asarray(inputs["b1"], np.float32)
    W2 = np.asarray(inputs["W2"], np.float32)
    b2 = np.asarray(inputs["b2"], np.float32)
    W3 = np.asarray(inputs["W3"], np.float32)
    b3 = np.asarray(inputs["b3"], np.float32)
    W_out = np.asarray(inputs["W_out"], np.float32)
    b_out = np.asarray(inputs["b_out"], np.float32)

    W1x = W1[:D].astype(np.float64)  # [128, 1024] rows feeding the dense bottom
    wx13 = (W_cont @ W1x) * (S_E * S_W)  # carry the fp8 product scale
    wx = np.zeros((N_CONT_PAD, L1), np.float32)
    wx[:N_CONT] = wx13.astype(np.float32)
    wx = np.ascontiguousarray(wx).astype(ml_dtypes.bfloat16)
    b1f = (b1.astype(np.float64) + b_cont @ W1x).astype(np.float32) * S_H
    W1e = W1[D:]  # [3328, 1024]

    w1e_sb = _q8(
        np.ascontiguousarray(
            W1e.reshape(N_TABLES, P, L1).transpose(1, 0, 2)
        ),
        S_W,
    )
    w2_sb = _q8(
        np.ascontiguousarray(W2.reshape(L1 // P, P, L2).transpose(1, 0, 2)), S_W
    )
    w3_sb = _q8(
        np.ascontiguousarray(W3.reshape(L2 // P, P, L3).transpose(1, 0, 2)), S_W
    )
    wo_sb = np.ascontiguousarray(
        W_out.reshape(L3 // P, P, 1).transpose(1, 0, 2).reshape(P, L3 // P)
    ).astype(ml_dtypes.bfloat16)
    b1_sb = np.ascontiguousarray(b1f.reshape(L1 // P, P).T)
    b2_sb = np.ascontiguousarray((b2 * S_H).reshape(L2 // P, P).T)
    b3_sb = np.ascontiguousarray(b3.reshape(L3 // P, P).T)
    bo_sb = np.ascontiguousarray(b_out.reshape(1, 1))

    return dict(
        emb=emb, w1e=w1e_sb, wx=wx, w2=w2_sb, w3=w3_sb, wo=wo_sb,
        b1=b1_sb, b2=b2_sb, b3=b3_sb, bo=bo_sb,
    )


def prep_core(inputs, core, vocab=VOCAB, shard=BATCH // N_CORES):
    """Per-core shard of indices and continuous features."""
    cf = np.asarray(inputs["continuous_features"], np.float32)
    cat = np.asarray(inputs["categorical_features"])
    n_bt = shard // P
    offs = (np.arange(N_TABLES, dtype=np.int64) * vocab)[None, :]
    sl = slice(core * shard, (core + 1) * shard)
    gidx = (cat[sl] + offs).astype(np.int32)  # [shard, 26]
    idx_c = np.ascontiguousarray(
        gidx.reshape(n_bt, P, N_TABLES).transpose(1, 0, 2).reshape(P, n_bt * N_TABLES)
    )
    xT_c = np.zeros((N_CONT_PAD, shard), np.float32)
    xT_c[:N_CONT] = cf[sl].T
    xT_c = np.ascontiguousarray(xT_c).astype(ml_dtypes.bfloat16)
    return dict(idx=idx_c, xT=xT_c)


_CACHE = {}


def get_nc(repeat=1):
    key = ("nc", repeat)
    if key not in _CACHE:
        _CACHE[key] = build_nc(repeat=repeat)
    return _CACHE[key]


def run(inputs, trace=False, repeat=1, **spmd_kwargs):
    from concourse.bass_utils import run_bass_kernel_spmd

    nc = get_nc(repeat)

    shared = prep_shared(inputs)
    in_maps = []
    for c in range(N_CORES):
        m = dict(shared)
        m.update(prep_core(inputs, c))
        in_maps.append(m)

    res = run_bass_kernel_spmd(
        nc, in_maps, core_ids=list(range(N_CORES)), trace=trace, **spmd_kwargs
    )
    ys = [r["y"].reshape(-1) for r in res.results]
    out = np.concatenate(ys).reshape(BATCH, 1).astype(np.float32)
    return out, res


def kernel(**inputs):
    out, _ = run(inputs, trace=False)
    return out
